# revision 1
# baseline (speedup 1.0000x reference)
"""DTNNStep (gnn message passing) on 8 Trainium2 NeuronCores.

Strategy (edge-parallel, per the sharding hint):
  * Edges (2M, sorted by membership_i) are sharded across 8 cores at atom
    boundaries: core c owns atoms [12500c, 12500(c+1)) and exactly the edges
    whose destination (membership_i) falls in that range.
  * Within a core, edges are split into 4 substreams by membership_j range
    (j in [25000k, 25000(k+1))) so that gather indices fit the int16 limit of
    the hardware dma_gather, and so the 4 substreams pack the 128-partition
    segmented scan.
  * Device per core:
      - atom_hidden table: a_h = atom_features @ W_cf + b_cf computed on
        device (bf16 in / f32 accum), stored as a bf16 [rows,128] DRAM table.
      - per 1024-edge tile per substream: distT matmul (weights stationary,
        bias folded via a ones row), transpose-mode dma_gather of a_h rows
        (F-major out), DVE multiply, W_fc matmul, ACT tanh into a packed
        [128,1024] tile (substream k at partitions 32k..32k+31), then one
        segmented scan (tensor_tensor_scan: state = mask*state + tanh) whose
        per-segment totals appear at segment-end columns.
      - fin = atom_features - tanh((b_df * a_h) @ W_fc) for the core's own
        atoms (f32).
  * Host: shards/pads inputs (layout only), then reads the scan output at
    (host-known) segment-end columns, adds the 4 substream partials and fin.
"""

import os
import sys

for _p in ("/opt/trn_rl_repo", "/root/.axon_site/_ro/trn_rl_repo"):
    if os.path.isdir(_p) and _p not in sys.path:
        sys.path.append(_p)

import numpy as np
from ml_dtypes import bfloat16
from contextlib import ExitStack

import concourse.bass as bass
import concourse.bacc as bacc
import concourse.mybir as mybir
import concourse.tile as tile
from concourse.bass_utils import run_bass_kernel_spmd

BF16 = mybir.dt.bfloat16
F32 = mybir.dt.float32
I16 = mybir.dt.int16


class Cfg:
    def __init__(self, n_atoms=100000, n_emb=30, n_dist=100, n_hid=60,
                 n_cores=8, n_sub=4, jrange=25000, c=1024, c2=500,
                 table_rows=100352, slab=8192, gather_chunk=512, jumbo=2048):
        self.n_atoms = n_atoms
        self.n_emb = n_emb
        self.n_dist = n_dist
        self.n_hid = n_hid
        self.n_cores = n_cores
        self.n_sub = n_sub
        self.jrange = jrange
        self.c = c              # pipeline tile columns (edges per substream-tile)
        self.c2 = c2            # fin-phase chunk
        self.apc = n_atoms // n_cores
        self.table_rows = table_rows  # multiple of 1024, >= n_atoms
        self.slab = slab        # atoms per a_fT slab DMA (multiple of 1024)
        self.gather_chunk = gather_chunk
        self.jumbo = jumbo      # columns per SWDGE bulk DMA (multiple of c)
        assert jumbo % c == 0
        assert table_rows % 1024 == 0 and table_rows >= n_atoms
        assert slab % 1024 == 0
        assert self.apc % c2 == 0
        assert jrange * n_sub >= n_atoms
        assert jrange <= 32767


DEFAULT_CFG = Cfg()


def build_program(cfg, cap):
    """Build + compile the (SPMD-identical) Bass program for one core."""
    c = cfg.c
    nt = cap // c
    assert cap % c == 0
    nd1 = cfg.n_dist + 1   # dist rows + ones row
    ne1 = cfg.n_emb + 1    # emb rows + ones row
    H, F = cfg.n_hid, cfg.n_emb

    nc = bacc.Bacc("TRN2", target_bir_lowering=False, debug=False,
                   num_devices=cfg.n_cores, num_swdge_queues=4)

    distT = nc.dram_tensor("distT", [cfg.n_sub, nd1, cap], BF16, kind="ExternalInput").ap()
    af_exp = nc.dram_tensor("af_exp", [cfg.n_sub, ne1, cap], BF16, kind="ExternalInput").ap()
    maskx = nc.dram_tensor("maskx", [128, cap], BF16, kind="ExternalInput").ap()
    a_fT_own = nc.dram_tensor("a_fT_own", [ne1, cfg.apc], BF16, kind="ExternalInput").ap()
    a_f_own = nc.dram_tensor("a_f_own", [cfg.n_emb, cfg.apc], F32, kind="ExternalInput").ap()
    Wdf = nc.dram_tensor("Wdf", [nd1, H], BF16, kind="ExternalInput").ap()
    Wcf = nc.dram_tensor("Wcf", [ne1, H], BF16, kind="ExternalInput").ap()
    Wfc = nc.dram_tensor("Wfc", [H, 32], BF16, kind="ExternalInput").ap()  # padded to 32 cols
    bdf = nc.dram_tensor("bdf", [H, 1], F32, kind="ExternalInput").ap()
    scanout = nc.dram_tensor("scanout", [128, cap], BF16, kind="ExternalOutput").ap()
    fin = nc.dram_tensor("fin", [cfg.n_emb, cfg.apc], F32, kind="ExternalOutput").ap()

    with tile.TileContext(nc) as tc, ExitStack() as ctx:
        wpool = ctx.enter_context(tc.tile_pool(name="weights", bufs=1))
        wdf_sb = wpool.tile([nd1, H], BF16)
        nc.sync.dma_start(wdf_sb[:], Wdf[:])
        wcf_sb = wpool.tile([ne1, H], BF16)
        nc.sync.dma_start(wcf_sb[:], Wcf[:])
        wfc_sb = wpool.tile([H, 32], BF16)
        nc.sync.dma_start(wfc_sb[:], Wfc[:])
        bdf_sb = wpool.tile([H, 1], F32)
        nc.sync.dma_start(bdf_sb[:], bdf[:])

        # ---------- edge pipeline -------------------------------------------
        # Bulk streams go through SWDGE (gpsimd) jumbo DMAs: HWDGE descriptors
        # all drain through SDMA engine 0 here, while SWDGE sprays across all
        # 16 engines. Jumbo = jb columns covering all 4 substreams per load.
        jb = cfg.jumbo
        nj = cap // jb
        tpj = jb // c
        distT_r = distT.rearrange("s r c -> r s c")
        af_exp_r = af_exp.rearrange("s r c -> r s c")
        with tc.tile_pool(name="ep_d", bufs=3) as dpool, \
             tc.tile_pool(name="ep_a", bufs=3) as apool, \
             tc.tile_pool(name="ep_h", bufs=3) as hpool, \
             tc.tile_pool(name="ep_pr", bufs=3) as prpool, \
             tc.tile_pool(name="ep_pk", bufs=2) as ppool, \
             tc.tile_pool(name="ep_mk", bufs=2) as mpool, \
             tc.tile_pool(name="ep_sc", bufs=2) as spool, \
             tc.tile_pool(name="ep_ps1", bufs=2, space="PSUM") as ps1, \
             tc.tile_pool(name="ep_ps2", bufs=1, space="PSUM") as ps2:
            carry = None
            for j in range(nj):
                # one DMA instruction per substream: each SWDGE instruction's
                # descriptors drain on a single SDMA engine, so splitting
                # spreads the load bandwidth across engines
                dj = dpool.tile([nd1, cfg.n_sub, jb], BF16, tag="dj")
                for k in range(cfg.n_sub):
                    nc.gpsimd.dma_start(dj[:, k, :],
                                        distT_r[:, k, j * jb:(j + 1) * jb])
                aj = apool.tile([ne1, cfg.n_sub, jb], BF16, tag="aj")
                for k in range(cfg.n_sub):
                    nc.gpsimd.dma_start(aj[:, k, :],
                                        af_exp_r[:, k, j * jb:(j + 1) * jb])
                mj_ = mpool.tile([128, jb], BF16, tag="mj")
                nc.gpsimd.dma_start(mj_[0:64, :], maskx[0:64, j * jb:(j + 1) * jb])
                nc.gpsimd.dma_start(mj_[64:128, :], maskx[64:128, j * jb:(j + 1) * jb])
                stg = spool.tile([128, jb], BF16, tag="stg")
                for tt in range(tpj):
                    c0 = tt * c
                    packed = ppool.tile([128, c], BF16, tag="packed")
                    prods = []
                    for k in range(cfg.n_sub):
                        psdh = ps1.tile([H, c], F32, tag="psdh", bufs=2)
                        psah = ps1.tile([H, c], F32, tag="psah", bufs=1)
                        for n0 in range(0, c, 512):
                            nn = min(512, c - n0)
                            nc.tensor.matmul(psdh[:, n0:n0 + nn], lhsT=wdf_sb[:],
                                             rhs=dj[:, k, c0 + n0:c0 + n0 + nn],
                                             start=True, stop=True)
                            nc.tensor.matmul(psah[:, n0:n0 + nn], lhsT=wcf_sb[:],
                                             rhs=aj[:, k, c0 + n0:c0 + n0 + nn],
                                             start=True, stop=True)
                        dh = hpool.tile([H, c], BF16, tag="dh")
                        nc.scalar.copy(dh[:], psdh[:])
                        prod = prpool.tile([H, c], BF16, tag="prod", bufs=6)
                        nc.vector.tensor_tensor(prod[:], dh[:], psah[:],
                                                op=mybir.AluOpType.mult)
                        prods.append(prod)
                    for n0 in range(0, c, 512):
                        nn = min(512, c - n0)
                        psoh = ps2.tile([128, 512], F32, tag="psoh", bufs=2)
                        for k in range(cfg.n_sub):
                            nc.tensor.matmul(psoh[32 * k:32 * k + 32, :nn],
                                             lhsT=wfc_sb[:],
                                             rhs=prods[k][:, n0:n0 + nn],
                                             start=True, stop=True,
                                             tile_position=(0, 32 * k))
                        nc.scalar.activation(packed[:, n0:n0 + nn], psoh[:, :nn],
                                             mybir.ActivationFunctionType.Tanh)
                    nc.vector.tensor_tensor_scan(
                        stg[:, c0:c0 + c], data0=mj_[:, c0:c0 + c],
                        data1=packed[:],
                        initial=(0.0 if carry is None else carry),
                        op0=mybir.AluOpType.mult, op1=mybir.AluOpType.add)
                    carry = stg[:, c0 + c - 1:c0 + c]
                # write-out on HWDGE so the in-order gpsimd queue stays a pure
                # load-prefetch stream (a gpsimd write here would block the
                # next jumbo's loads behind this jumbo's last scan)
                nc.sync.dma_start(scanout[:, j * jb:(j + 1) * jb], stg[:])

        # ---------- phase 3: fin = a_f - tanh((b_df*a_h) @ W_fc) ------------
        with tc.tile_pool(name="fi_in", bufs=1) as fpool, \
             tc.tile_pool(name="fi_s", bufs=3) as s2, \
             tc.tile_pool(name="fi_ps", bufs=4, space="PSUM") as p2:
            afo = fpool.tile([ne1, cfg.apc], BF16)
            nc.sync.dma_start(afo[:], a_fT_own[:])
            aff = fpool.tile([cfg.n_emb, cfg.apc], F32)
            nc.sync.dma_start(aff[:], a_f_own[:])
            for q0 in range(0, cfg.apc, cfg.c2):
                psii = p2.tile([H, cfg.c2], F32, tag="psii")
                nc.tensor.matmul(psii[:], lhsT=wcf_sb[:], rhs=afo[:, q0:q0 + cfg.c2],
                                 start=True, stop=True)
                pii = s2.tile([H, cfg.c2], BF16, tag="pii")
                nc.scalar.mul(pii[:], psii[:], bdf_sb[:, 0:1])
                psf = p2.tile([F, cfg.c2], F32, tag="psf")
                nc.tensor.matmul(psf[:], lhsT=wfc_sb[:, 0:F], rhs=pii[:],
                                 start=True, stop=True)
                th2 = s2.tile([F, cfg.c2], F32, tag="th2")
                nc.scalar.activation(th2[:], psf[:],
                                     mybir.ActivationFunctionType.Tanh)
                fn = s2.tile([F, cfg.c2], F32, tag="fn")
                nc.vector.tensor_tensor(fn[:], aff[:, q0:q0 + cfg.c2], th2[:],
                                        op=mybir.AluOpType.subtract)
                nc.sync.dma_start(fin[:, q0:q0 + cfg.c2], fn[:])

    nc.compile()
    return nc


def host_prep(inputs, cfg):
    """Shard + lay out inputs for the 8 cores. Returns (in_maps, post_data, cap)."""
    af = np.asarray(inputs["atom_features"], dtype=np.float32)
    dist = np.asarray(inputs["distance"], dtype=np.float32)
    mi = np.asarray(inputs["distance_membership_i"]).astype(np.int64)
    mj = np.asarray(inputs["distance_membership_j"]).astype(np.int64)
    W_cf = np.asarray(inputs["W_cf"], dtype=np.float32)
    W_df = np.asarray(inputs["W_df"], dtype=np.float32)
    W_fc = np.asarray(inputs["W_fc"], dtype=np.float32)
    b_cf = np.asarray(inputs["b_cf"], dtype=np.float32)
    b_df = np.asarray(inputs["b_df"], dtype=np.float32)

    n_emb, n_dist, H = cfg.n_emb, cfg.n_dist, cfg.n_hid
    c = cfg.c

    Wdf_aug = np.vstack([W_df, b_df[None, :]]).astype(bfloat16)
    Wcf_aug = np.vstack([W_cf, b_cf[None, :]]).astype(bfloat16)
    Wfc_pad = np.zeros((H, 32), np.float32)
    Wfc_pad[:, :n_emb] = W_fc
    Wfc_pad = Wfc_pad.astype(bfloat16)
    bdf_col = b_df[:, None].astype(np.float32)

    af_aug = np.concatenate([af, np.ones((cfg.n_atoms, 1), np.float32)], axis=1
                            ).astype(bfloat16)  # [n_atoms, n_emb+1]

    bounds = np.searchsorted(mi, np.arange(0, cfg.n_atoms + 1, cfg.apc))
    core_sels = []
    max_n = 0
    for cid in range(cfg.n_cores):
        e0, e1 = bounds[cid], bounds[cid + 1]
        kk = mj[e0:e1] // cfg.jrange
        sels = [e0 + np.nonzero(kk == k)[0] for k in range(cfg.n_sub)]
        core_sels.append(sels)
        max_n = max(max_n, max(len(s) for s in sels))
    jb = cfg.jumbo
    cap = max(jb, ((max_n + jb - 1) // jb) * jb)
    nt = cap // c

    in_maps = []
    post_data = []
    for cid in range(cfg.n_cores):
        A0 = cid * cfg.apc
        sels = core_sels[cid]
        distT = np.zeros((cfg.n_sub, n_dist + 1, cap), bfloat16)
        af_exp = np.zeros((cfg.n_sub, n_emb + 1, cap), bfloat16)
        maskx = np.ones((128, cap), np.float32)
        ends_k = []
        for k in range(cfg.n_sub):
            sel = sels[k]
            n = len(sel)
            if n:
                distT[k, :n_dist, :n] = dist[sel].T.astype(bfloat16)
                distT[k, n_dist, :n] = bfloat16(1.0)
                af_exp[k, :, :n] = af_aug[mj[sel]].T
                ids = mi[sel] - A0
                m = np.ones(cap, np.float32)
                m[0] = 0.0
                m[1:n][ids[1:] != ids[:-1]] = 0.0
                maskx[32 * k:32 * k + n_emb, :] = m[None, :]
                endpos = np.nonzero(np.r_[ids[1:] != ids[:-1], True])[0]
                ends_k.append((endpos.astype(np.int64), ids[endpos].astype(np.int64)))
            else:
                ends_k.append((np.zeros(0, np.int64), np.zeros(0, np.int64)))
        in_maps.append(dict(
            distT=distT,
            af_exp=af_exp,
            maskx=maskx.astype(bfloat16),
            a_fT_own=np.ascontiguousarray(af_aug[A0:A0 + cfg.apc].T),
            a_f_own=np.ascontiguousarray(af[A0:A0 + cfg.apc].T.astype(np.float32)),
            Wdf=Wdf_aug, Wcf=Wcf_aug, Wfc=Wfc_pad, bdf=bdf_col,
        ))
        post_data.append(ends_k)
    return in_maps, post_data, cap


def host_post(results, post_data, cfg):
    out = np.empty((cfg.n_atoms, cfg.n_emb), np.float32)
    for cid in range(cfg.n_cores):
        r = results[cid]
        agg = np.asarray(r["fin"]).astype(np.float32).T.copy()  # [apc, n_emb]
        sc = np.asarray(r["scanout"])  # bf16 [128, cap]
        for k in range(cfg.n_sub):
            endpos, atoms = post_data[cid][k]
            if len(endpos):
                vals = sc[32 * k:32 * k + cfg.n_emb][:, endpos].astype(np.float32)
                np.add.at(agg, atoms, vals.T)
        out[cid * cfg.apc:(cid + 1) * cfg.apc] = agg
    return out


_CACHE = {}


def kernel(**inputs):
    cfg = DEFAULT_CFG
    in_maps, post_data, cap = host_prep(inputs, cfg)
    if cap not in _CACHE:
        _CACHE[cap] = build_program(cfg, cap)
    nc = _CACHE[cap]
    res = run_bass_kernel_spmd(nc, in_maps, core_ids=list(range(cfg.n_cores)))
    return host_post(res.results, post_data, cfg)



# revision 2
# speedup vs baseline: 1.8424x; 1.8424x over previous
"""DTNNStep (gnn message passing) on 8 Trainium2 NeuronCores.

Strategy (edge-parallel, per the sharding hint):
  * Edges (2M, sorted by membership_i) are sharded across 8 cores at atom
    boundaries: core c owns atoms [12500c, 12500(c+1)) and exactly the edges
    whose destination (membership_i) falls in that range.
  * Within a core, edges are split into 4 substreams by membership_j range
    so 4 substreams pack the 128-partition segmented scan; substreams are
    processed in PAIRS to fill the PE array / DVE lanes.
  * Device pipeline per 512-col chunk (2048 edges = 4 substreams x 512):
      - dh: 4 matmuls (lhsT=Wdf_aug [101,64] bf16, rhs=centered distance
        in f8e3 [101,512]) -> psdh pair tiles [128,512] (s0 at part 0-63,
        s1 at 64-127 via tile_position).  Distance is shipped as
        (d - 0.5) in float8_e3m4; 0.5*sum_k W_df[k] is folded into the
        bias row on host, so the quantization error is zero-mean.
      - ah: 2 block-diagonal matmuls (lhsT=Wcf2 [61,128]: two Wcf blocks
        + bias row, rhs=af pair tile bf16 [61,512]) -> psah [128,512].
      - ACT copies psah -> SBUF bf16 (the only PSUM exit copy).
      - DVE mult: prod = ah_sb (SBUF bf16) * psdh (PSUM f32) -> bf16.
      - fc: 2 matmuls (lhsT=Wfc2 [128,64]: block structure contracting the
        pair's two 60-row H blocks) -> psfc [128,512] (F rows at
        0-29/30-59/64-93/94-123).
      - mask: 1 matmul (lhsT=IND [4,128] indicator, rhs=mask4 [4,512]) ->
        psmask [128,512]: broadcasts 4 host-built segment-boundary mask
        rows to their 30-row blocks (replaces a 16 MB mask DMA stream).
      - ACT tanh psfc -> packed SBUF bf16.
      - DVE segmented scan: state = psmask*state + packed; per-segment
        totals appear at segment-end columns (host-known positions).
  * Host: shards/pads inputs (layout only), reads the scan output at
    segment-end columns, adds the 4 substream partials and the
    fin = atom_features - tanh((b_df * atom_hidden) @ W_fc) correction
    computed on-device in a small trailing phase.
"""

import os
import sys

for _p in ("/opt/trn_rl_repo", "/root/.axon_site/_ro/trn_rl_repo"):
    if os.path.isdir(_p) and _p not in sys.path:
        sys.path.append(_p)

import numpy as np
from ml_dtypes import bfloat16, float8_e3m4
from contextlib import ExitStack

import concourse.bass as bass
import concourse.bacc as bacc
import concourse.mybir as mybir
import concourse.tile as tile
from concourse.bass_utils import run_bass_kernel_spmd

BF16 = mybir.dt.bfloat16
F8E3 = mybir.dt.float8e3
F32 = mybir.dt.float32

# partition row-base of each substream's 30 output rows in the packed tile
ROWBASE = (0, 30, 64, 94)


class Cfg:
    def __init__(self, n_atoms=100000, n_emb=30, n_dist=100, n_hid=60,
                 n_cores=8, n_sub=4, jrange=25000, c=512, c2=500,
                 jumbo=2048):
        self.n_atoms = n_atoms
        self.n_emb = n_emb
        self.n_dist = n_dist
        self.n_hid = n_hid
        self.n_cores = n_cores
        self.n_sub = n_sub
        self.jrange = jrange
        self.c = c              # pipeline chunk columns
        self.c2 = c2            # fin-phase chunk
        self.apc = n_atoms // n_cores
        self.jumbo = jumbo      # columns per SWDGE bulk DMA (multiple of c)
        assert jumbo % c == 0
        assert self.apc % c2 == 0
        assert jrange * n_sub >= n_atoms


DEFAULT_CFG = Cfg()


def build_program(cfg, cap):
    """Build + compile the (SPMD-identical) Bass program for one core."""
    c = cfg.c
    assert cap % cfg.jumbo == 0
    nd1 = cfg.n_dist + 1   # dist rows + ones row
    ne1 = cfg.n_emb + 1    # emb rows + ones row (fin phase)
    H, F = cfg.n_hid, cfg.n_emb

    nc = bacc.Bacc("TRN2", target_bir_lowering=False, debug=False,
                   num_devices=cfg.n_cores, num_swdge_queues=4)

    distT = nc.dram_tensor("distT", [cfg.n_sub, nd1, cap], F8E3, kind="ExternalInput").ap()
    af2 = nc.dram_tensor("af2", [2, H + 1, cap], BF16, kind="ExternalInput").ap()
    mask4 = nc.dram_tensor("mask4", [cfg.n_sub, cap], BF16, kind="ExternalInput").ap()
    a_fT_own = nc.dram_tensor("a_fT_own", [ne1, cfg.apc], BF16, kind="ExternalInput").ap()
    a_f_own = nc.dram_tensor("a_f_own", [cfg.n_emb, cfg.apc], F32, kind="ExternalInput").ap()
    Wdf = nc.dram_tensor("Wdf", [nd1, 64], BF16, kind="ExternalInput").ap()
    Wcf2 = nc.dram_tensor("Wcf2", [H + 1, 128], BF16, kind="ExternalInput").ap()
    Wfc2 = nc.dram_tensor("Wfc2", [128, 64], BF16, kind="ExternalInput").ap()
    IND = nc.dram_tensor("IND", [cfg.n_sub, 128], BF16, kind="ExternalInput").ap()
    Wcf = nc.dram_tensor("Wcf", [ne1, H], BF16, kind="ExternalInput").ap()
    Wfc = nc.dram_tensor("Wfc", [H, 32], BF16, kind="ExternalInput").ap()
    bdf = nc.dram_tensor("bdf", [H, 1], F32, kind="ExternalInput").ap()
    scanout = nc.dram_tensor("scanout", [128, cap], BF16, kind="ExternalOutput").ap()
    fin = nc.dram_tensor("fin", [cfg.n_emb, cfg.apc], F32, kind="ExternalOutput").ap()

    with tile.TileContext(nc) as tc, ExitStack() as ctx:
        wpool = ctx.enter_context(tc.tile_pool(name="weights", bufs=1))
        wdf_sb = wpool.tile([nd1, 64], BF16)
        nc.sync.dma_start(wdf_sb[:], Wdf[:])
        wcf2_sb = wpool.tile([H + 1, 128], BF16)
        nc.sync.dma_start(wcf2_sb[:], Wcf2[:])
        wfc2_sb = wpool.tile([128, 64], BF16)
        nc.sync.dma_start(wfc2_sb[:], Wfc2[:])
        ind_sb = wpool.tile([cfg.n_sub, 128], BF16)
        nc.sync.dma_start(ind_sb[:], IND[:])
        wcf_sb = wpool.tile([ne1, H], BF16)
        nc.sync.dma_start(wcf_sb[:], Wcf[:])
        wfc_sb = wpool.tile([H, 32], BF16)
        nc.sync.dma_start(wfc_sb[:], Wfc[:])
        bdf_sb = wpool.tile([H, 1], F32)
        nc.sync.dma_start(bdf_sb[:], bdf[:])

        # ---------- edge pipeline -------------------------------------------
        jb = cfg.jumbo
        nj = cap // jb
        tpj = jb // c
        with tc.tile_pool(name="ep_d", bufs=3) as dpool, \
             tc.tile_pool(name="ep_a", bufs=3) as apool, \
             tc.tile_pool(name="ep_m4", bufs=3) as m4pool, \
             tc.tile_pool(name="ep_ah", bufs=3) as ahpool, \
             tc.tile_pool(name="ep_pr", bufs=3) as prpool, \
             tc.tile_pool(name="ep_pk", bufs=3) as pkpool, \
             tc.tile_pool(name="ep_sc", bufs=2) as spool, \
             tc.tile_pool(name="ep_psd", bufs=2, space="PSUM") as psd, \
             tc.tile_pool(name="ep_psa", bufs=2, space="PSUM") as psa, \
             tc.tile_pool(name="ep_psf", bufs=2, space="PSUM") as psf, \
             tc.tile_pool(name="ep_psm", bufs=2, space="PSUM") as psm:
            carry = None
            for j in range(nj):
                j0 = j * jb
                dj = dpool.tile([nd1, cfg.n_sub, jb], F8E3, tag="dj")
                for k in range(cfg.n_sub):
                    nc.gpsimd.dma_start(dj[:, k, :],
                                        distT.rearrange("s r c -> r s c")[:, k, j0:j0 + jb])
                aj = apool.tile([H + 1, 2, jb], BF16, tag="aj")
                for p in range(2):
                    nc.gpsimd.dma_start(aj[:, p, :],
                                        af2.rearrange("s r c -> r s c")[:, p, j0:j0 + jb])
                m4 = m4pool.tile([cfg.n_sub, jb], BF16, tag="m4")
                nc.gpsimd.dma_start(m4[:], mask4[:, j0:j0 + jb])
                stg = spool.tile([128, jb], BF16, tag="stg")
                for tt in range(tpj):
                    c0 = tt * c
                    # ---- dh: 4 matmuls into 2 pair tiles --------------------
                    psdh = []
                    for p in range(2):
                        t_ = psd.tile([128, c], F32, tag=f"psdh{p}", bufs=1)
                        for h in range(2):
                            s = 2 * p + h
                            nc.tensor.matmul(t_[64 * h:64 * h + 64, :],
                                             lhsT=wdf_sb[:],
                                             rhs=dj[:, s, c0:c0 + c],
                                             start=True, stop=True,
                                             tile_position=(0, 64 * h))
                        psdh.append(t_)
                    # ---- ah: 1 block-diag matmul per pair, ACT copy out -----
                    ahs = []
                    for p in range(2):
                        t_ = psa.tile([128, c], F32, tag=f"psah{p}", bufs=1)
                        nc.tensor.matmul(t_[:], lhsT=wcf2_sb[:],
                                         rhs=aj[:, p, c0:c0 + c],
                                         start=True, stop=True)
                        ah = ahpool.tile([128, c], BF16, tag=f"ah{p}", bufs=2)
                        nc.scalar.copy(ah[:], t_[:])
                        ahs.append(ah)
                    # ---- prod = ah * dh (DVE, psum operand) -----------------
                    prods = []
                    for p in range(2):
                        pr = prpool.tile([128, c], BF16, tag=f"prod{p}", bufs=2)
                        nc.vector.tensor_tensor(pr[:], ahs[p][:], psdh[p][:],
                                                op=mybir.AluOpType.mult)
                        prods.append(pr)
                    # ---- fc + mask matmuls ----------------------------------
                    pfc = psf.tile([128, c], F32, tag="psfc", bufs=2)
                    for p in range(2):
                        nc.tensor.matmul(pfc[64 * p:64 * p + 64, :],
                                         lhsT=wfc2_sb[:], rhs=prods[p][:],
                                         start=True, stop=True,
                                         tile_position=(0, 64 * p))
                    pmask = psm.tile([128, c], F32, tag="psmask", bufs=2)
                    nc.tensor.matmul(pmask[:], lhsT=ind_sb[:],
                                     rhs=m4[:, c0:c0 + c], start=True, stop=True)
                    # ---- tanh + segmented scan ------------------------------
                    packed = pkpool.tile([128, c], BF16, tag="packed")
                    nc.scalar.activation(packed[:], pfc[:],
                                         mybir.ActivationFunctionType.Tanh)
                    nc.vector.tensor_tensor_scan(
                        stg[:, c0:c0 + c], data0=pmask[:], data1=packed[:],
                        initial=(0.0 if carry is None else carry),
                        op0=mybir.AluOpType.mult, op1=mybir.AluOpType.add)
                    carry = stg[:, c0 + c - 1:c0 + c]
                nc.sync.dma_start(scanout[:, j0:j0 + jb], stg[:])

        # ---------- fin = a_f - tanh((b_df*a_h) @ W_fc) ---------------------
        with tc.tile_pool(name="fi_in", bufs=1) as fpool, \
             tc.tile_pool(name="fi_s", bufs=3) as s2, \
             tc.tile_pool(name="fi_ps", bufs=4, space="PSUM") as p2:
            afo = fpool.tile([ne1, cfg.apc], BF16)
            nc.sync.dma_start(afo[:], a_fT_own[:])
            aff = fpool.tile([cfg.n_emb, cfg.apc], F32)
            nc.sync.dma_start(aff[:], a_f_own[:])
            for q0 in range(0, cfg.apc, cfg.c2):
                psii = p2.tile([H, cfg.c2], F32, tag="psii")
                nc.tensor.matmul(psii[:], lhsT=wcf_sb[:], rhs=afo[:, q0:q0 + cfg.c2],
                                 start=True, stop=True)
                pii = s2.tile([H, cfg.c2], BF16, tag="pii")
                nc.scalar.mul(pii[:], psii[:], bdf_sb[:, 0:1])
                psff = p2.tile([F, cfg.c2], F32, tag="psf")
                nc.tensor.matmul(psff[:], lhsT=wfc_sb[:, 0:F], rhs=pii[:],
                                 start=True, stop=True)
                th2 = s2.tile([F, cfg.c2], F32, tag="th2")
                nc.scalar.activation(th2[:], psff[:],
                                     mybir.ActivationFunctionType.Tanh)
                fn = s2.tile([F, cfg.c2], F32, tag="fn")
                nc.vector.tensor_tensor(fn[:], aff[:, q0:q0 + cfg.c2], th2[:],
                                        op=mybir.AluOpType.subtract)
                nc.sync.dma_start(fin[:, q0:q0 + cfg.c2], fn[:])

    nc.compile()
    return nc


def host_prep(inputs, cfg):
    """Shard + lay out inputs for the 8 cores. Returns (in_maps, post_data, cap)."""
    af = np.asarray(inputs["atom_features"], dtype=np.float32)
    dist = np.asarray(inputs["distance"], dtype=np.float32)
    mi = np.asarray(inputs["distance_membership_i"]).astype(np.int64)
    mj = np.asarray(inputs["distance_membership_j"]).astype(np.int64)
    W_cf = np.asarray(inputs["W_cf"], dtype=np.float32)
    W_df = np.asarray(inputs["W_df"], dtype=np.float32)
    W_fc = np.asarray(inputs["W_fc"], dtype=np.float32)
    b_cf = np.asarray(inputs["b_cf"], dtype=np.float32)
    b_df = np.asarray(inputs["b_df"], dtype=np.float32)

    n_emb, n_dist, H = cfg.n_emb, cfg.n_dist, cfg.n_hid

    # dh weights: [101, 64], cols 60-63 zero.  Distance is shipped centered
    # (d - 0.5), so fold 0.5*colsum(W_df) into the bias row.
    Wdf_aug = np.zeros((n_dist + 1, 64), np.float32)
    Wdf_aug[:n_dist, :H] = W_df
    Wdf_aug[n_dist, :H] = b_df + 0.5 * W_df.sum(axis=0)
    Wdf_aug = Wdf_aug.astype(bfloat16)

    # ah weights: block-diag [61, 128]: cols 0-59 <- rows 0-29 (s_even),
    # cols 64-123 <- rows 30-59 (s_odd); bias row 60 on both blocks.
    Wcf2 = np.zeros((H + 1, 128), np.float32)
    Wcf2[0:n_emb, 0:H] = W_cf
    Wcf2[n_emb:2 * n_emb, 64:64 + H] = W_cf
    Wcf2[H, 0:H] = b_cf
    Wcf2[H, 64:64 + H] = b_cf
    Wcf2 = Wcf2.astype(bfloat16)

    # fc weights: [128, 64]: cols 0-29 contract prod rows 0-59 (s_even),
    # cols 30-59 contract prod rows 64-123 (s_odd).
    Wfc2 = np.zeros((128, 64), np.float32)
    Wfc2[0:H, 0:n_emb] = W_fc
    Wfc2[64:64 + H, n_emb:2 * n_emb] = W_fc
    Wfc2 = Wfc2.astype(bfloat16)

    # mask broadcast indicator [4, 128]: row k -> ROWBASE[k]..+30
    INDm = np.zeros((cfg.n_sub, 128), np.float32)
    for k in range(cfg.n_sub):
        INDm[k, ROWBASE[k]:ROWBASE[k] + n_emb] = 1.0
    INDm = INDm.astype(bfloat16)

    # fin-phase weights
    Wcf_aug = np.vstack([W_cf, b_cf[None, :]]).astype(bfloat16)
    Wfc_pad = np.zeros((H, 32), np.float32)
    Wfc_pad[:, :n_emb] = W_fc
    Wfc_pad = Wfc_pad.astype(bfloat16)
    bdf_col = b_df[:, None].astype(np.float32)

    af_aug = np.concatenate([af, np.ones((cfg.n_atoms, 1), np.float32)], axis=1
                            ).astype(bfloat16)  # [n_atoms, n_emb+1]
    af_bf = af.astype(bfloat16)  # [n_atoms, n_emb]

    bounds = np.searchsorted(mi, np.arange(0, cfg.n_atoms + 1, cfg.apc))
    core_sels = []
    max_n = 0
    for cid in range(cfg.n_cores):
        e0, e1 = bounds[cid], bounds[cid + 1]
        kk = mj[e0:e1] // cfg.jrange
        sels = [e0 + np.nonzero(kk == k)[0] for k in range(cfg.n_sub)]
        core_sels.append(sels)
        max_n = max(max_n, max(len(s) for s in sels))
    jb = cfg.jumbo
    cap = max(jb, ((max_n + jb - 1) // jb) * jb)

    dist_q = (dist - np.float32(0.5)).astype(float8_e3m4)  # centered e3m4

    in_maps = []
    post_data = []
    for cid in range(cfg.n_cores):
        A0 = cid * cfg.apc
        sels = core_sels[cid]
        distT = np.zeros((cfg.n_sub, n_dist + 1, cap), float8_e3m4)
        af2 = np.zeros((2, H + 1, cap), bfloat16)
        mask4 = np.ones((cfg.n_sub, cap), np.float32)
        ends_k = []
        for k in range(cfg.n_sub):
            sel = sels[k]
            n = len(sel)
            if n:
                distT[k, :n_dist, :n] = dist_q[sel].T
                distT[k, n_dist, :n] = float8_e3m4(1.0)
                p, h = divmod(k, 2)
                af2[p, h * n_emb:(h + 1) * n_emb, :n] = af_bf[mj[sel]].T
                ids = mi[sel] - A0
                m = np.ones(cap, np.float32)
                m[0] = 0.0
                m[1:n][ids[1:] != ids[:-1]] = 0.0
                mask4[k] = m
                endpos = np.nonzero(np.r_[ids[1:] != ids[:-1], True])[0]
                ends_k.append((endpos.astype(np.int64), ids[endpos].astype(np.int64)))
            else:
                ends_k.append((np.zeros(0, np.int64), np.zeros(0, np.int64)))
        af2[:, H, :] = bfloat16(1.0)  # ones row for the ah bias
        in_maps.append(dict(
            distT=distT,
            af2=af2,
            mask4=mask4.astype(bfloat16),
            a_fT_own=np.ascontiguousarray(af_aug[A0:A0 + cfg.apc].T),
            a_f_own=np.ascontiguousarray(af[A0:A0 + cfg.apc].T.astype(np.float32)),
            Wdf=Wdf_aug, Wcf2=Wcf2, Wfc2=Wfc2, IND=INDm,
            Wcf=Wcf_aug, Wfc=Wfc_pad, bdf=bdf_col,
        ))
        post_data.append(ends_k)
    return in_maps, post_data, cap


def host_post(results, post_data, cfg):
    out = np.empty((cfg.n_atoms, cfg.n_emb), np.float32)
    for cid in range(cfg.n_cores):
        r = results[cid]
        agg = np.asarray(r["fin"]).astype(np.float32).T.copy()  # [apc, n_emb]
        sc = np.asarray(r["scanout"])  # bf16 [128, cap]
        for k in range(cfg.n_sub):
            endpos, atoms = post_data[cid][k]
            if len(endpos):
                rb = ROWBASE[k]
                vals = sc[rb:rb + cfg.n_emb][:, endpos].astype(np.float32)
                np.add.at(agg, atoms, vals.T)
        out[cid * cfg.apc:(cid + 1) * cfg.apc] = agg
    return out


_CACHE = {}


def kernel(**inputs):
    cfg = DEFAULT_CFG
    in_maps, post_data, cap = host_prep(inputs, cfg)
    if cap not in _CACHE:
        _CACHE[cap] = build_program(cfg, cap)
    nc = _CACHE[cap]
    res = run_bass_kernel_spmd(nc, in_maps, core_ids=list(range(cfg.n_cores)))
    return host_post(res.results, post_data, cfg)
